# revision 74
# baseline (speedup 1.0000x reference)
"""Bass/Tile kernel for nn_CausalSelfAttention (GQA + RMS-norm + RoPE + sliding window).

Sharding: 4-way sequence x 2-way heads over 8 NeuronCores.
Per core: 1024 queries, 8 q-heads, 2 kv-heads, kv buffer of 2048 rows
(1024-row halo to the left, zero-padded for the first sequence shard).

All layouts are transpose-free on device:
  - host passes x^T and W^T slices
  - projections produce q^T/k^T [hd, seq] (lhsT = W tile) and v [seq, hd]
    (lhsT = x^T tile) directly
  - scores^T [sk, sq] = k_tile^T.T @ q^T ; PV: y^T += v_tile.T @ probs^T
  - out^T = Wo^T.T @ y^T  (partial over this core's heads; host sums pairs)

RMS-norm scales fold into the RoPE multiply; 1/sqrt(hd) folds into the
q-side scale; softmax needs no running max (rms-normed logits bounded by
sqrt(128)). Causal/window edges are handled by multiplying probs with
host-precomputed 0/1 mask tiles on DVE (cheap, keeps Pool free); the
sliding-window structure means only 4 of 6 sk-tile pairs per query block
need a mask. Padded halo keys project to v=0 so they never pollute y;
only the softmax denominator needs the pz zeroing on the first shard.

All projections (Q/K/V) run as 3-term split-fp8 DoubleRow matmuls:
host ships x = hi+lo and W*64 = hi+lo in e4m3 (row-interleaved packs,
one DMA per tile) and the device drops the lo*lo term.  DoubleRow
processes two 128-deep contraction tiles per instruction at 0.5
cycles/column, so each projection costs 0.75x its bf16 column count
while hi+lo carries ~9 mantissa bits (slightly better than bf16).
The *64 weight scale cancels inside the rms-norm (eps rescaled 64^2)
and is divided out of the v-path on the host after the pair-sum.

Attention runs per 128-query block as a 2-head depth-2 software
pipeline: each step issues the NEXT pair's score matmuls before the
current pair's PV matmuls, so the PE never stalls on the
Act-exp -> DVE-mask chain.  Wo stays resident in SBUF (loads deferred
behind the latency-critical prologue DMAs); rms-norm partition
reduction runs on the PE (ones matmul) with two-stage drains emitted
behind the next group's matmuls.  y is split per kv-group (yt2) so
out-proj doesn't false-depend on the other group's last write, and
attention block 4 is emitted before out-proj block 0 so the out-proj
never waits on the softmax DVE tail.  The final out-proj tile is
drained in two halves to shorten the exposed copy+DMA tail.
"""

import sys

if "/opt/trn_rl_repo" not in sys.path:
    sys.path.insert(0, "/opt/trn_rl_repo")

import ml_dtypes
import numpy as np

import concourse.mybir as mybir
import concourse.tile as tile
from concourse import bacc, bass_isa, bass_utils

f32 = mybir.dt.float32
f32r = mybir.dt.float32r
bf16 = mybir.dt.bfloat16
f8 = mybir.dt.float8e4
DR = mybir.MatmulPerfMode.DoubleRow
AF = mybir.ActivationFunctionType
W8SCALE = 64.0  # host scales Wq/Wk into fp8 range; cancels in rms-norm
                # (eps rescaled 64^2)
WVSCALE = 32.0  # Wv scale: smaller so y' = WVSCALE*y stays inside e4m3
WOSCALE = 64.0  # Wo scale; host divides the output by WVSCALE*WOSCALE

D = 2048
S = 4096
NH = 16
NKV = 4
HD = 128
SEQW = 4
HEADW = 2
SQ = S // SEQW              # 1024 queries per core
HALO = 1024                 # local window
KVLEN = SQ + HALO           # 2048
QH = NH // HEADW            # 8 q-heads per core
KVH = NKV // HEADW          # 2 kv-heads per core
NB = 512                    # matmul moving block
NSQB = SQ // NB             # 2
NDT = D // 128              # 16
NKT = KVLEN // 128          # 16
EPS = 1.1920929e-07


def build_program():
    nc = bacc.Bacc(
        "TRN2",
        target_bir_lowering=False,
        debug=False,
        enable_asserts=False,
        num_devices=8,
    )
    xtkv_p = nc.dram_tensor("xtkv_p", [4, D, 2, NB], f8, kind="ExternalInput").ap()
    wqt_p = nc.dram_tensor("wqt_p", [2, D, 2, 4 * HD], f8, kind="ExternalInput").ap()
    wkt_p = nc.dram_tensor("wkt_p", [D, 2, KVH * HD], f8, kind="ExternalInput").ap()
    wvt_p = nc.dram_tensor("wvt_p", [D, 2, KVH * HD], f8, kind="ExternalInput").ap()
    wot8_d = nc.dram_tensor("wot8", [QH * HD, 2, D], f8, kind="ExternalInput").ap()
    ckd = nc.dram_tensor("ck", [128, KVLEN], bf16, kind="ExternalInput").ap()
    skd = nc.dram_tensor("sk", [128, KVLEN], bf16, kind="ExternalInput").ap()
    pzcd = nc.dram_tensor("pzc", [128, 8], f32, kind="ExternalInput").ap()
    mskd = nc.dram_tensor(
        "msk", [128, 8, 128], bf16, kind="ExternalInput"
    ).ap()
    outT = nc.dram_tensor("outT", [D, SQ], bf16, kind="ExternalOutput").ap()

    with tile.TileContext(nc) as tc:
        with (
            tc.tile_pool(name="persist", bufs=1) as persist,
            tc.tile_pool(name="scratch", bufs=4) as sc,
        ):
            # --- constants ---
            ones128 = persist.tile([128, 128], bf16)
            nc.vector.memset(ones128, 1.0)
            eps_q = persist.tile([128, 1], f32)
            nc.vector.memset(eps_q, 128.0 * EPS * W8SCALE * W8SCALE)
            eps_k = persist.tile([128, 1], f32)
            nc.vector.memset(eps_k, EPS * W8SCALE * W8SCALE)
            pzc_sb = persist.tile([128, 8], f32)

            ck_sb = persist.tile([128, KVLEN], bf16)
            sk_sb = persist.tile([128, KVLEN], bf16)
            # packed hi/lo fp8 weights: [part, d-tile, {hi,lo}, width]
            wkp_sb = persist.tile([128, NDT, 2, KVH * HD], f8)
            wvp_sb = persist.tile([128, NDT, 2, KVH * HD], f8)
            msk_sb = persist.tile([128, 8, 128], bf16)
            # [p, y-row-pair, head-in-pair, {hi,lo}, d]
            wot8_sb = persist.tile([128, 4, 2, 2, D], f8)

            # split per 512-query block: q-block1 drains run after
            # attention blocks 0-3 are emitted, and deps are tile-granular
            qrot_b = [
                persist.tile([128, QH, NB], bf16, name=f"qrot_b{b}")
                for b in range(NSQB)
            ]
            krot4 = [
                persist.tile([128, KVH, NB], bf16, name=f"krot{qq}")
                for qq in range(4)
            ]
            v4 = [
                persist.tile([128, 4, KVH * HD], bf16, name=f"v4_{qq}")
                for qq in range(4)
            ]
            # per-kv-group y in split fp8 (hi+lo) for the DoubleRow out-proj
            yt8h = [
                persist.tile([128, 4, SQ], f8, name=f"yt8h_{g}")
                for g in range(KVH)
            ]
            yt8l = [
                persist.tile([128, 4, SQ], f8, name=f"yt8l_{g}")
                for g in range(KVH)
            ]

            def drain_stage1(acc, s_scale, s_bias, nm, psA, artag="ar"):
                """acc: PSUM [128, NB] raw projection.  Short chain so the PE
                partition-reduce never waits long: raw copy (Act) -> square
                (DVE) -> ones128 matmul (PE) -> sqrt (Act)."""
                raw = sc.tile([128, NB], bf16, tag="big0", name=f"raw{nm}")
                nc.scalar.copy(out=raw, in_=acc)
                sqd_t = sc.tile([128, NB], bf16, tag="big1", name=f"sqd{nm}")
                nc.vector.tensor_mul(out=sqd_t, in0=raw, in1=raw)
                allr = psA.tile([128, NB], f32, tag=artag, name=f"allr{nm}")
                nc.tensor.matmul(
                    allr, lhsT=ones128, rhs=sqd_t, start=True, stop=True
                )
                s_full = sc.tile([128, NB], f32, tag="big3", name=f"sf{nm}")
                nc.scalar.activation(
                    out=s_full, in_=allr, func=AF.Sqrt, bias=s_bias, scale=s_scale
                )
                return raw, s_full

            def drain_stage2(st, out_slice, ctab, stab, nm):
                """Norm+rope tail; runs on DVE/Pool, overlapping the next
                matmul stream."""
                raw, s_full = st
                a_full = sc.tile([128, NB], bf16, tag="big4", name=f"af{nm}")
                with nc.allow_low_precision(reason="f32r is 4-byte fp32 storage"):
                    nc.vector.reciprocal(out=a_full, in_=s_full)
                rawa = sc.tile([128, NB], bf16, tag="big5", name=f"ra{nm}")
                nc.vector.tensor_mul(out=rawa, in0=raw, in1=a_full)
                t1 = sc.tile([128, NB], bf16, tag="big2", name=f"t1{nm}")
                nc.vector.tensor_mul(out=t1, in0=rawa, in1=ctab)
                t2 = sc.tile([128, NB], bf16, tag="big1", name=f"t2{nm}")
                nc.vector.tensor_mul(out=t2, in0=rawa, in1=stab)
                usw = sc.tile([128, NB], bf16, tag="big0", name=f"usw{nm}")
                nc.gpsimd.tensor_copy(out=usw[0:64, :], in_=t2[64:128, :])
                nc.gpsimd.tensor_copy(out=usw[64:128, :], in_=t2[0:64, :])
                nc.vector.tensor_add(
                    out=out_slice[0:64, :], in0=t1[0:64, :], in1=usw[0:64, :]
                )
                nc.vector.tensor_sub(
                    out=out_slice[64:128, :], in0=t1[64:128, :], in1=usw[64:128, :]
                )

            # ====== Projection phase: kv quarters + q blocks interleaved ======
            # Emission order qtr0,1,2, Qblk0, qtr3, Qblk1 so attention's
            # dependencies (krot/v then qrot) drain as early as possible.
            # Per quarter: k-stream (2 accs) then v-stream (4 accs) so k accs
            # complete and drain while v matmuls still run.  Q blocks reuse
            # the quarter's resident x tiles and run heads in 2 groups of 4.
            with tc.tile_pool(name="xs", bufs=6) as xs:
              with tc.tile_pool(name="psP", bufs=6, space="PSUM") as psP, \
                 tc.tile_pool(name="psA", bufs=2, space="PSUM") as psA:
                xts = {}

                def load_xtile(qtr, dc):
                    # packed hi (t=0) + lo (t=1) in one DMA
                    t = xs.tile(
                        [128, 4, 2, NB], f8, tag="xk", bufs=8,
                        name=f"xk{qtr}_{dc}",
                    )
                    nc.sync.dma_start(
                        out=t,
                        in_=xtkv_p[
                            qtr, 512 * dc : 512 * (dc + 1), :, :
                        ].rearrange("(c p) t w -> p c t w", p=128),
                    )
                    xts[(qtr, dc)] = t

                def emit_qtr(qtr):
                    if qtr == 0:
                        # cold start: strictly order DMAs by first-use time
                        for dc in range(4):
                            load_xtile(0, dc)
                            nc.sync.dma_start(
                                out=wkp_sb[:, 4 * dc : 4 * (dc + 1), :, :],
                                in_=wkt_p[
                                    512 * dc : 512 * (dc + 1), :, :
                                ].rearrange("(c p) t w -> p c t w", p=128),
                            )
                        for vc in range(2):
                            nc.sync.dma_start(
                                out=wvp_sb[:, 8 * vc : 8 * (vc + 1), :, :],
                                in_=wvt_p[
                                    1024 * vc : 1024 * (vc + 1), :, :
                                ].rearrange("(c p) t w -> p c t w", p=128),
                            )
                        nc.sync.dma_start(out=ck_sb, in_=ckd)
                        nc.sync.dma_start(out=sk_sb, in_=skd)
                        nc.sync.dma_start(out=msk_sb, in_=mskd)
                        nc.sync.dma_start(out=pzc_sb, in_=pzcd)
                    def k_group(qtr, kvh):
                        # term-major (hi@hi sweep first) so the cold-start
                        # DMA queue only needs hi tiles for the first sweep
                        kacc = psP.tile(
                            [128, NB], f32, tag="acc", name=f"kacc{qtr}_{kvh}"
                        )
                        hs = slice(HD * kvh, HD * (kvh + 1))
                        for term in range(3):
                            wti = 1 if term == 1 else 0
                            xti = 1 if term == 2 else 0
                            for dc in range(4):
                                xt = xts[(qtr, dc)]
                                for dl in (0, 2):
                                    d = 4 * dc + dl
                                    nc.tensor.matmul(
                                        kacc,
                                        lhsT=wkp_sb[:, d : d + 2, wti, hs],
                                        rhs=xt[:, dl : dl + 2, xti, :],
                                        start=(term == 0 and d == 0),
                                        stop=(term == 2 and d == NDT - 2),
                                        perf_mode=DR,
                                    )
                        return kacc

                    def v_group(qtr, lt):
                        vacc = psP.tile(
                            [128, KVH * HD], f32, tag="acc", name=f"vacc{qtr}_{lt}"
                        )
                        ls = slice(128 * lt, 128 * (lt + 1))
                        for term in range(3):
                            wti = 1 if term == 2 else 0
                            xti = 1 if term == 1 else 0
                            for dc in range(4):
                                xt = xts[(qtr, dc)]
                                for dl in (0, 2):
                                    d = 4 * dc + dl
                                    nc.tensor.matmul(
                                        vacc,
                                        lhsT=xt[:, dl : dl + 2, xti, ls],
                                        rhs=wvp_sb[:, d : d + 2, wti, :],
                                        start=(term == 0 and d == 0),
                                        stop=(term == 2 and d == NDT - 2),
                                        perf_mode=DR,
                                    )
                        return vacc

                    # Emission: each acc group runs to completion, its drain
                    # chain (Act/DVE/PE-ones) emitted behind the NEXT group's
                    # matmuls so the PE never waits on a drain dependency.
                    kacc0 = k_group(qtr, 0)
                    if qtr < 3:
                        for dc in range(4):
                            load_xtile(qtr + 1, dc)
                    kacc1 = k_group(qtr, 1)
                    vacc0 = v_group(qtr, 0)
                    kst0 = drain_stage1(
                        kacc0, 1.0 / 128.0, eps_k, f"k{qtr}_0", psA
                    )
                    vacc1 = v_group(qtr, 1)
                    kst1 = drain_stage1(
                        kacc1, 1.0 / 128.0, eps_k, f"k{qtr}_1", psA
                    )
                    drain_stage2(
                        kst0,
                        krot4[qtr][:, 0, :],
                        ck_sb[:, NB * qtr : NB * (qtr + 1)],
                        sk_sb[:, NB * qtr : NB * (qtr + 1)],
                        f"k{qtr}_0",
                    )
                    vacc2 = v_group(qtr, 2)
                    nc.scalar.copy(out=v4[qtr][:, 0, :], in_=vacc0)
                    drain_stage2(
                        kst1,
                        krot4[qtr][:, 1, :],
                        ck_sb[:, NB * qtr : NB * (qtr + 1)],
                        sk_sb[:, NB * qtr : NB * (qtr + 1)],
                        f"k{qtr}_1",
                    )
                    vacc3 = v_group(qtr, 3)
                    nc.scalar.copy(out=v4[qtr][:, 1, :], in_=vacc1)
                    nc.scalar.copy(out=v4[qtr][:, 2, :], in_=vacc2)
                    nc.scalar.copy(out=v4[qtr][:, 3, :], in_=vacc3)


                wq_tiles = {}

                def load_wq(blk, hg, dc):
                    wq2 = xs.tile(
                        [128, 4, 2, 4 * HD], f8, tag="wqd", bufs=5,
                        name=f"wqd{blk}_{hg}_{dc}",
                    )
                    nc.sync.dma_start(
                        out=wq2,
                        in_=wqt_p[
                            hg, 512 * dc : 512 * (dc + 1), :, :
                        ].rearrange("(c p) t w -> p c t w", p=128),
                    )
                    wq_tiles[(blk, hg, dc)] = wq2

                def emit_qblk(blk, hgs=(0, 1), gs=4, accpool=None,
                              acctag="acc", arpool=None, artag="ar",
                              preload_next=False):
                    qtr = 2 + blk
                    wqts = {
                        (hg, dc): wq_tiles[(blk, hg, dc)]
                        for hg in hgs
                        for dc in range(4)
                        if (blk, hg, dc) in wq_tiles
                    }

                    def q_group(hg, hh):
                        acc = (accpool or psP).tile(
                            [128, NB], f32, tag=acctag,
                            name=f"qacc{blk}_{hg}_{hh}",
                        )
                        hsl = slice(HD * hh, HD * (hh + 1))
                        for term in range(3):
                            wti = 1 if term == 1 else 0
                            xti = 1 if term == 2 else 0
                            for dc in range(4):
                                wq2 = wqts[(hg, dc)]
                                xt = xts[(qtr, dc)]
                                for dl in (0, 2):
                                    d = 4 * dc + dl
                                    nc.tensor.matmul(
                                        acc,
                                        lhsT=wq2[:, dl : dl + 2, wti, hsl],
                                        rhs=xt[:, dl : dl + 2, xti, :],
                                        start=(term == 0 and d == 0),
                                        stop=(term == 2 and d == NDT - 2),
                                        perf_mode=DR,
                                    )
                        return acc

                    def ds1(st):
                        hg, hh, acc = st
                        return drain_stage1(
                            acc, 1.0, eps_q, f"q{blk}_{hg}_{hh}",
                            arpool or psA, artag,
                        )

                    def ds2(st, s1):
                        hg, hh, _ = st
                        h = gs * hg + hh
                        drain_stage2(
                            s1,
                            qrot_b[blk][:, h, :],
                            ck_sb[:, HALO + NB * blk : HALO + NB * (blk + 1)],
                            sk_sb[:, HALO + NB * blk : HALO + NB * (blk + 1)],
                            f"q{blk}_{hg}_{hh}",
                        )

                    for dc in range(4):
                        if (hgs[0], dc) not in wqts:
                            load_wq(blk, hgs[0], dc)
                            wqts[(hgs[0], dc)] = wq_tiles[(blk, hgs[0], dc)]
                    groups = [(hg, hh) for hg in hgs for hh in range(gs)]
                    pend = []  # [(st, s1_or_None), ...] staggered drains
                    for gi, (hg, hh) in enumerate(groups):
                        acc = q_group(hg, hh)
                        if gi == 0 and len(hgs) > 1:
                            for dc in range(4):
                                if (hgs[1], dc) not in wqts:
                                    load_wq(blk, hgs[1], dc)
                                    wqts[(hgs[1], dc)] = wq_tiles[
                                        (blk, hgs[1], dc)
                                    ]
                        if pend:
                            st, s1 = pend[-1]
                            if s1 is None:
                                pend[-1] = (st, ds1(st))
                        if len(pend) > 1:
                            st, s1 = pend.pop(0)
                            ds2(st, s1)
                        pend.append(((hg, hh, acc), None))
                    pend = [
                        (st, s1 if s1 is not None else ds1(st))
                        for st, s1 in pend
                    ]
                    if preload_next:
                        for dc in range(4):
                            load_wq(blk + 1, 0, dc)
                    for st, s1 in pend:
                        ds2(st, s1)

                emit_qtr(0)
                emit_qtr(1)
                emit_qtr(2)
                emit_qtr(3)
                emit_qblk(0, preload_next=True)
                emit_qblk(1)
                # out-proj weights first needed ~60us later; DMA is idle
                # during early attention
                for yp in range(4):
                    nc.sync.dma_start(
                        out=wot8_sb[:, yp, :, :, :],
                        in_=wot8_d[
                            256 * yp : 256 * (yp + 1), :, :
                        ].rearrange("(hin p) t d -> p hin t d", p=128),
                    )

              # ============ Phase A + O: attention, then out-proj ============
              # 256-query attention blocks: each needs only 10 sk tiles
              # (vs 12 per 512-block) thanks to the sliding window.
              QB = 256
              with tc.tile_pool(name="probs", bufs=6) as pp, tc.tile_pool(
                name="psY", bufs=3, space="PSUM"
            ) as psY, tc.tile_pool(
                name="psR", bufs=1, space="PSUM"
            ) as psR, tc.tile_pool(
                name="psS", bufs=2, space="PSUM"
            ) as psS:
                def attn_block(qb, kvh_order=(0, 1), split_fast=False):
                    # 128-query block qb (0..7), both kv heads; 4 q-heads of a
                    # kv group fused into single wide matmuls/exps.  9 sk
                    # tiles: 4 pairs + 1 singleton (the causal edge).
                    qsl = slice(128 * qb, 128 * (qb + 1))
                    for kvh in kvh_order:
                        h0 = 4 * kvh
                        nmg = f"{qb}_{kvh}"
                        yacc4 = psY.tile(
                            [128, 4, 128], f32, tag="y", name=f"y{nmg}"
                        )
                        racc = psR.tile([128, 256], f32, tag="r", name=f"rc{nmg}")
                        rsumA = pp.tile(
                            [128, 2, 128], bf16, tag="rs", bufs=3, name=f"rs{nmg}"
                        )
                        qrh = qrot_b[qb // 4][
                            :, h0 : h0 + 4, 128 * (qb % 4) : 128 * (qb % 4 + 1)
                        ]

                        def scj(j):
                            if j < 4:
                                t = psS.tile(
                                    [128, 8, 128], f32, tag="s", name=f"sa{nmg}_{j}"
                                )
                                for jj in range(2):
                                    kt = qb + 2 * j + jj
                                    nc.tensor.matmul(
                                        t[:, 4 * jj : 4 * jj + 4, :],
                                        lhsT=krot4[kt // 4][
                                            :,
                                            kvh,
                                            128 * (kt % 4) : 128 * (kt % 4 + 1),
                                        ],
                                        rhs=qrh,
                                        start=True,
                                        stop=True,
                                    )
                            else:
                                t = psS.tile(
                                    [128, 4, 128], f32, tag="s", name=f"sa{nmg}_4"
                                )
                                kt = qb + 8
                                nc.tensor.matmul(
                                    t,
                                    lhsT=krot4[kt // 4][
                                        :, kvh, 128 * (kt % 4) : 128 * (kt % 4 + 1)
                                    ],
                                    rhs=qrh,
                                    start=True,
                                    stop=True,
                                )
                            return t

                        def em(j, t):
                            if j < 4:
                                pt = pp.tile(
                                    [128, 8, 128], bf16, tag="pt", bufs=6,
                                    name=f"pt{nmg}_{j}",
                                )
                            else:
                                pt = pp.tile(
                                    [128, 4, 128], bf16, tag="pt", bufs=6,
                                    name=f"pt{nmg}_4",
                                )
                            nc.scalar.activation(
                                out=pt, in_=t, func=AF.Exp, bias=0.0, scale=1.0
                            )
                            if j == 0:
                                nc.vector.tensor_mul(
                                    out=pt[:, 0:4, :],
                                    in0=pt[:, 0:4, :],
                                    in1=msk_sb[:, 0:4, :],
                                )
                            elif j == 4:
                                nc.vector.tensor_mul(
                                    out=pt, in0=pt, in1=msk_sb[:, 4:8, :]
                                )
                            return pt

                        def pv(j, pt):
                            if j < 4:
                                for jj in range(2):
                                    tt = 2 * j + jj
                                    kt = qb + tt
                                    nc.tensor.matmul(
                                        yacc4,
                                        lhsT=v4[kt // 4][:, kt % 4, HD * kvh : HD * (kvh + 1)],
                                        rhs=pt[:, 4 * jj : 4 * jj + 4, :],
                                        start=(tt == 0),
                                        stop=False,
                                    )
                                    nc.tensor.matmul(
                                        racc,
                                        lhsT=ones128,
                                        rhs=pt[:, 4 * jj + 2 : 4 * jj + 4, :],
                                        start=(tt == 0),
                                        stop=False,
                                    )
                                if j == 0:
                                    nc.vector.tensor_add(
                                        out=rsumA,
                                        in0=pt[:, 0:2, :],
                                        in1=pt[:, 4:6, :],
                                    )
                                else:
                                    tmp = pp.tile(
                                        [128, 2, 128], bf16, tag="rt", bufs=3,
                                        name=f"rt{nmg}_{j}",
                                    )
                                    nc.vector.tensor_add(
                                        out=tmp,
                                        in0=pt[:, 0:2, :],
                                        in1=pt[:, 4:6, :],
                                    )
                                    nc.vector.tensor_add(
                                        out=rsumA, in0=rsumA, in1=tmp
                                    )
                            else:
                                kt = qb + 8
                                nc.tensor.matmul(
                                    yacc4,
                                    lhsT=v4[kt // 4][:, kt % 4, HD * kvh : HD * (kvh + 1)],
                                    rhs=pt,
                                    start=False,
                                    stop=True,
                                )
                                nc.tensor.matmul(
                                    racc,
                                    lhsT=ones128,
                                    rhs=pt[:, 2:4, :],
                                    start=False,
                                    stop=True,
                                )
                                nc.vector.tensor_add(
                                    out=rsumA, in0=rsumA, in1=pt[:, 0:2, :]
                                )

                        # depth-2 pipeline over 4 pairs + singleton
                        sacc, ptb = {}, {}
                        sacc[0] = scj(0)
                        sacc[1] = scj(1)
                        ptb[0] = em(0, sacc.pop(0))
                        for j in range(5):
                            if j + 2 <= 4:
                                sacc[j + 2] = scj(j + 2)
                            if j + 1 <= 4:
                                ptb[j + 1] = em(j + 1, sacc.pop(j + 1))
                            pv(j, ptb.pop(j))

                        # heads h0+2/h0+3 via the PE racc; then its psR slot
                        # frees for heads h0/h0+1's partition-reduce
                        nc.vector.tensor_scalar_sub(
                            out=racc, in0=racc,
                            scalar1=pzc_sb[:, qb : qb + 1],
                        )
                        rbB = sc.tile([128, 256], f32r, tag="big5", name=f"rbB{nmg}")
                        with nc.allow_low_precision(reason="f32r 4-byte"):
                            nc.vector.reciprocal(out=rbB, in_=racc)
                        ysc = pp.tile(
                            [128, 4, 128], bf16, tag="ysc", bufs=3,
                            name=f"ysc{nmg}",
                        )
                        for i in range(2):
                            nc.vector.tensor_mul(
                                out=ysc[:, 2 + i, :],
                                in0=yacc4[:, 2 + i, :],
                                in1=rbB[:, 128 * i : 128 * (i + 1)],
                            )
                        if split_fast:
                            nc.scalar.copy(
                                out=yt8h[kvh][:, 2:4, qsl],
                                in_=ysc[:, 2:4, :],
                            )
                            nc.vector.tensor_sub(
                                out=yt8l[kvh][:, 2:4, qsl],
                                in0=ysc[:, 2:4, :],
                                in1=yt8h[kvh][:, 2:4, qsl],
                            )
                        rallA = psR.tile([128, 256], f32, tag="r", name=f"rlA{nmg}")
                        nc.tensor.matmul(
                            rallA, lhsT=ones128, rhs=rsumA, start=True, stop=True
                        )
                        nc.vector.tensor_scalar_sub(
                            out=rallA, in0=rallA,
                            scalar1=pzc_sb[:, qb : qb + 1],
                        )
                        rinvA = sc.tile(
                            [128, 256], f32r, tag="big5", name=f"rbA{nmg}"
                        )
                        with nc.allow_low_precision(reason="f32r 4-byte"):
                            nc.vector.reciprocal(out=rinvA, in_=rallA)
                        for i in range(2):
                            nc.vector.tensor_mul(
                                out=ysc[:, i, :],
                                in0=yacc4[:, i, :],
                                in1=rinvA[:, 128 * i : 128 * (i + 1)],
                            )
                        # split y' into fp8 hi + lo for the DoubleRow
                        # out-proj; Pool (gpsimd) is idle during attention.
                        # The final block uses Act/DVE (idle right after)
                        # so out-proj 1 isn't gated on Pool queue latency.
                        if split_fast:
                            nc.scalar.copy(
                                out=yt8h[kvh][:, 0:2, qsl],
                                in_=ysc[:, 0:2, :],
                            )
                            nc.vector.tensor_sub(
                                out=yt8l[kvh][:, 0:2, qsl],
                                in0=ysc[:, 0:2, :],
                                in1=yt8h[kvh][:, 0:2, qsl],
                            )
                        else:
                            nc.gpsimd.tensor_copy(
                                out=yt8h[kvh][:, :, qsl], in_=ysc
                            )
                            nc.gpsimd.tensor_sub(
                                out=yt8l[kvh][:, :, qsl],
                                in0=ysc,
                                in1=yt8h[kvh][:, :, qsl],
                            )

                def outproj(bo):
                  # ---- out-proj for this 512-block (weights resident) ----
                  # psum->sbuf copies on Act (idle here); DVE is busy with
                  # the neighbouring attention phase's softmax tail
                  for dm in range(NDT):
                        oacc = psS.tile(
                            [128, NB], f32, tag="s", name=f"oacc{dm}_{bo}"
                        )
                        ds = slice(128 * dm, 128 * (dm + 1))
                        bs = slice(NB * bo, NB * (bo + 1))
                        yps = (0, 1, 2, 3) if bo == 0 else (2, 3, 1, 0)
                        for yi, yp in enumerate(yps):
                            kvh, hp = yp // 2, yp % 2
                            hsl = slice(2 * hp, 2 * hp + 2)
                            rh = yt8h[kvh][:, hsl, bs]
                            rl = yt8l[kvh][:, hsl, bs]
                            nc.tensor.matmul(
                                oacc, lhsT=wot8_sb[:, yp, :, 0, ds], rhs=rh,
                                start=(yi == 0), stop=False, perf_mode=DR,
                            )
                            nc.tensor.matmul(
                                oacc, lhsT=wot8_sb[:, yp, :, 1, ds], rhs=rh,
                                start=False, stop=False, perf_mode=DR,
                            )
                            nc.tensor.matmul(
                                oacc, lhsT=wot8_sb[:, yp, :, 0, ds], rhs=rl,
                                start=False, stop=(yi == 3), perf_mode=DR,
                            )
                        ot = sc.tile([128, NB], bf16, tag="big0", name=f"ot{dm}_{bo}")
                        if bo == NSQB - 1 and dm == NDT - 1:
                            # split the very last chain so the exposed
                            # copy+DMA tail is as short as possible
                            for hv in range(2):
                                hs = slice(256 * hv, 256 * (hv + 1))
                                nc.scalar.copy(out=ot[:, hs], in_=oacc[:, hs])
                                nc.sync.dma_start(
                                    out=outT[
                                        128 * dm : 128 * (dm + 1),
                                        NB * bo + 256 * hv : NB * bo + 256 * (hv + 1),
                                    ],
                                    in_=ot[:, hs],
                                )
                        else:
                            nc.scalar.copy(out=ot, in_=oacc)
                            nc.sync.dma_start(
                                out=outT[
                                    128 * dm : 128 * (dm + 1),
                                    NB * bo : NB * (bo + 1),
                                ],
                                in_=ot,
                            )

                for qb in range(4):
                    attn_block(qb)
                attn_block(4)
                outproj(0)
                for qb in range(5, 7):
                    attn_block(qb)
                # last block converts kv-group 1's y first so out-proj 1
                # (which contracts kv-group 1 first) never waits on it
                attn_block(7, kvh_order=(1, 0), split_fast=True)
                outproj(1)

    nc.compile()
    return nc


def _split8(a):
    """f32 [R, C] -> row-interleaved [R, 2, C] float8_e4m3 (hi, lo)."""
    hi = a.astype(ml_dtypes.float8_e4m3)
    lo = (a - hi.astype(np.float32)).astype(ml_dtypes.float8_e4m3)
    return np.ascontiguousarray(np.stack([hi, lo], axis=1))


def _split8_chunked(a, nchunk):
    """f32 [R, nchunk*W] -> [nchunk, R, 2, W] fp8 (hi, lo), column-chunked."""
    r, c = a.shape
    w = c // nchunk
    hi = a.astype(ml_dtypes.float8_e4m3)
    lo = (a - hi.astype(np.float32)).astype(ml_dtypes.float8_e4m3)
    out = np.stack([hi, lo], axis=1)  # [R, 2, C]
    out = out.reshape(r, 2, nchunk, w).transpose(2, 0, 1, 3)
    return np.ascontiguousarray(out)


def host_prep(x, Wq, Wk, Wv, Wo):
    x2 = np.asarray(x, dtype=np.float32).reshape(S, D)
    xT = np.ascontiguousarray(x2.T)
    WqT = np.ascontiguousarray(np.asarray(Wq, np.float32).T) * W8SCALE
    WkT = np.ascontiguousarray(np.asarray(Wk, np.float32).T) * W8SCALE
    WvT = np.ascontiguousarray(np.asarray(Wv, np.float32).T) * WVSCALE
    WoT = np.ascontiguousarray(np.asarray(Wo, np.float32).T) * WOSCALE

    pos = np.arange(-HALO, S, dtype=np.float32)
    invf = 1.0 / (10000.0 ** (np.arange(0, HD, 2, dtype=np.float32) / HD))
    fr = pos[:, None] * invf[None, :]
    cosT = np.cos(fr).T.astype(np.float32)
    sinT = np.sin(fr).T.astype(np.float32)
    C2 = np.ascontiguousarray(np.concatenate([cosT, cosT], axis=0))
    S2 = np.ascontiguousarray(np.concatenate([sinT, sinT], axis=0))

    in_maps = []
    for si in range(SEQW):
        lo = si * SQ - HALO
        xtkv = np.zeros((D, KVLEN), np.float32)
        lo_c = max(lo, 0)
        xtkv[:, lo_c - lo :] = xT[:, lo_c : si * SQ + SQ]
        cks = C2[:, HALO + lo : HALO + lo + KVLEN].astype(ml_dtypes.bfloat16)
        sks = S2[:, HALO + lo : HALO + lo + KVLEN].astype(ml_dtypes.bfloat16)
        pz = np.full((128, 1), 0.0 if si == 0 else 1.0, np.float32)

        # 0/1 prob masks for the window/causal edge sk-tile pairs, with the
        # first-shard halo baked in.  msk[p, 4*blk+im, jj*NB+c] corresponds to
        # key 128*(4*blk+2*ip+jj)+p and query 512*blk+c of this shard.
        # edge-tile prob masks for 128-query blocks in the 4-head-fused
        # layout: slots 0-3 window tile t=0 (x4 heads), 4-7 causal tile t=8.
        # Patterns are block-independent; first-shard halo baked in.  Interior
        # halo tiles on the first shard are NOT masked: their probs are
        # exp(0)=1 with v=0, so only the softmax denominator needs fixing,
        # via the per-block halo count pzc.
        msk = np.zeros((128, 8, 128), np.float32)
        p_i = np.arange(128)
        c_i = np.arange(128)
        for im, t in enumerate((0, 0, 0, 0, 8, 8, 8, 8)):
            k_abs = si * SQ - HALO + 128 * t + p_i[:, None]
            q_abs = si * SQ + c_i[None, :]
            msk[:, im, :] = (
                (k_abs <= q_abs) & (k_abs > q_abs - HALO) & (k_abs >= 0)
            )
        msk_b = msk.astype(ml_dtypes.bfloat16)
        pzc = np.zeros((128, 8), np.float32)
        if si == 0:
            for qb in range(8):
                pzc[:, qb] = 128.0 * max(0, 7 - qb)

        xtkv_8 = _split8_chunked(xtkv, 4)
        for hi in range(HEADW):
            in_maps.append(
                dict(
                    xtkv_p=xtkv_8,
                    wqt_p=_split8_chunked(WqT[:, 1024 * hi : 1024 * (hi + 1)], 2),
                    wkt_p=_split8(WkT[:, 256 * hi : 256 * (hi + 1)]),
                    wvt_p=_split8(WvT[:, 256 * hi : 256 * (hi + 1)]),
                    wot8=_split8(WoT[1024 * hi : 1024 * (hi + 1), :]),
                    ck=cks,
                    sk=sks,
                    pzc=pzc,
                    msk=msk_b,
                )
            )
    return in_maps


def host_post(results):
    out = np.empty((S, D), np.float32)
    inv = 1.0 / (WVSCALE * WOSCALE)  # v-path and Wo scales
    for si in range(SEQW):
        acc = results[2 * si]["outT"].astype(np.float32) + results[
            2 * si + 1
        ]["outT"].astype(np.float32)
        out[si * SQ : (si + 1) * SQ, :] = acc.T * inv
    return out.reshape(1, S, D)


_cached_nc = None


def get_nc():
    global _cached_nc
    if _cached_nc is None:
        _cached_nc = build_program()
    return _cached_nc


def kernel(**inputs):
    nc = get_nc()
    in_maps = host_prep(
        inputs["x"], inputs["Wq"], inputs["Wk"], inputs["Wv"], inputs["Wo"]
    )
    res = bass_utils.run_bass_kernel_spmd(nc, in_maps, core_ids=list(range(8)))
    return host_post(res.results)

